# revision 57
# baseline (speedup 1.0000x reference)
"""Bass/Trainium2 kernel for nn_ClassQueryAttention.

Math (per batch b, x flattened to [C=256, N=16384]):
  logitsT[n,k] = x[:,n]^T qk[:,k]            (qk = (qe@Wk)^T/sqrt(D); per-k bias cancels in softmax)
  pT = exp(logitsT)                          (logits ~ N(0,1): no max-subtraction needed)
  y[k,c] = sum_n pT[n,k] x[c,n]  (flash-style PSUM accum; s[k] = sum_n pT[n,k] via ones column)
  xa[c]  = sum_k y[k,c]/s[k]
  gate   = (Wo@Wv) @ xa + K*(Wo@bv + bo)
  out[c,n] = x[c,n] * gate[c]

Single streamed pass: per 128-px window, logitsT comes from a matmul with the
x-chunk as the STATIONARY operand (21 moving cols instead of 256), so exp's
output is already in pT layout (no p transpose) at full 128-partition ACT
utilization. x is transposed per-window on the PE (bf16, 1 cyc/col); the
PSUM->SBUF copies convert xT to fp8e4 and alternate DVE/ACT. exp writes pT
as fp8e4 directly (padded to 32B/window for the DoubleRow weight-stride
rule). The y/s matmuls then run in fp8 DoubleRow mode: one matmul contracts
a PAIR of windows (lhsT [128,2,21], rhs [128,2,256]), halving PE time vs
bf16 (~10us saved/core; fp8 costs ~1e-2 rel err vs the 2e-2 gate, dominated
by xT quantization). y+s accumulate in one PSUM bank over all 128 windows,
so there is no softmax barrier; phase B is O(K*C) tiny, phase C multiplies
in-place and streams out.

Measured no-gos: DMA XBAR transpose runs ~100 GB/s on hw (5x slower than PE
transposes, and it head-of-line blocks its DMA queue); PE "pre-warm" dummy
matmuls and early/split tile-0 DMAs both LOSE time (the PE p-state ramp
rewards a late-but-dense schedule); compile-schedule variance between
near-identical programs is ~±4 us, so buffer counts (lt_ps=3, sm_ps=1) were
chosen by measurement.

HBM I/O is bf16 (host converts): 8 MiB in + 8 MiB out per core.
Sharding: data-parallel over batch B=8, one batch per NeuronCore, no collectives.
"""

import sys
from contextlib import ExitStack

import numpy as np
import ml_dtypes

sys.path.insert(0, "/opt/trn_rl_repo")

import concourse.bass as bass  # noqa: E402
import concourse.tile as tile  # noqa: E402
from concourse import bacc, mybir  # noqa: E402
from concourse.bass_utils import run_bass_kernel_spmd  # noqa: E402

B, C, HW = 8, 256, 128 * 128
K, D = 21, 256
P = 128          # partition count / channel chunk
NB = 2048        # big-tile pixels (per-tile DMA granularity)
NW = 128         # window pixels (transpose / pT granularity)
F32 = mybir.dt.float32
BF16 = mybir.dt.bfloat16
F8 = mybir.dt.float8e4
DR = mybir.MatmulPerfMode.DoubleRow
AF = mybir.ActivationFunctionType
BF = ml_dtypes.bfloat16

N_TILES = HW // NB           # 8
W_PER_T = NB // NW           # 16 windows per big tile
G_PER_T = W_PER_T // 4       # 4 transpose groups (4 windows each) per big tile
N_XBAR = 0                   # XBAR DMA transpose measured ~100GB/s on hw: off


def _body(ctx: ExitStack, tc: tile.TileContext, x, qk, m2, g0, idb, idf, out, sfx=""):
    nc = tc.nc

    def pool(name, **kw):
        return ctx.enter_context(tc.tile_pool(name=name + sfx, **kw))

    consts = pool("consts", bufs=1)
    qk0 = consts.tile([P, K], BF16, tag="qk0")
    qk1 = consts.tile([P, K], BF16, tag="qk1")
    m2t0 = consts.tile([P, C], F32, tag="m2t0")
    m2t1 = consts.tile([P, C], F32, tag="m2t1")
    g0_sb = consts.tile([P, 2], F32, tag="g0")
    idb_sb = consts.tile([P, P], BF16, tag="idb")
    idf_sb = consts.tile([1, 1], F32, tag="idf")


    # qk + idb are needed by the very first window's matmuls; m2t/g0/idf are
    # phase-B-only and issued after the x stream (end of pass A loop below).
    # DoubleRow weight APs need a pair stride of >=32 elems -> pad K=21 to 32
    ones_sb = consts.tile([P, 2, 32], F8, tag="ones")

    nc.sync.dma_start(qk0[:], qk[0:P, :])
    nc.sync.dma_start(qk1[:], qk[P : 2 * P, :])
    nc.sync.dma_start(idb_sb[:], idb[:, :])
    nc.vector.memset(ones_sb[:], 1.0)



    xbig = pool("xbig", bufs=1)
    lt_ps = pool("lt_ps", bufs=3, space="PSUM")     # logitsT [128, 16*21] f32
    xt_ps = pool("xt_ps", bufs=3, space="PSUM")     # xT group [128, 4, 256] bf16
    y_ps = pool("y_ps", bufs=1, space="PSUM")       # y+s accum [21, 257] f32
    pt_sb = pool("pt_sb", bufs=2)
    xt_sb = pool("xt_sb", bufs=8)
    smalls = pool("smalls", bufs=1)
    sm_ps = pool("sm_ps", bufs=1, space="PSUM")

    y = y_ps.tile([K, C + 1], F32, tag="y")
    nc.vector.memset(y[:], 0.0)

    # round-robin engine for the xT PSUM->SBUF copies (GPSIMD cannot read PSUM)
    cp_eng = [nc.vector.tensor_copy, nc.scalar.copy]
    cp_i = 0

    xres = {}
    pts = {}
    xts = {}

    def emit_y(t):
        """y/s matmuls for big tile t (pT + xT copies of tile t are complete).

        fp8 DoubleRow: each matmul contracts a PAIR of 128-px windows at
        0.5 cyc/output-row (lhsT [128, 2, K], rhs [128, 2, C])."""
        pt = pts[t]
        for w2 in range(W_PER_T // 2):
            gw2 = t * (W_PER_T // 2) + w2
            last = gw2 == HW // (2 * NW) - 1
            lhs = pt[:, 2 * w2 : 2 * w2 + 2, 0:K]
            rhs = xts[t, w2 // 2][:, 2 * (w2 % 2) : 2 * (w2 % 2) + 2, :]
            # start=False always: the bank is pre-zeroed by the memset above.
            # Two start=True groups in one bank reset each other (the w0 s-
            # matmul's start wiped the w0 y contribution).
            nc.tensor.matmul(
                y[:, 0:C], lhs, rhs,
                start=False, stop=last, perf_mode=DR, skip_group_check=True,
            )
            nc.tensor.matmul(
                y[:, C : C + 1], lhs, ones_sb[:, :, 0:1],
                start=False, stop=last, perf_mode=DR, skip_group_check=True,
            )

    # ---------------- Pass A: stream tiles; accumulate y/s in PSUM ----------
    for t in range(N_TILES):
        xb0 = xbig.tile([P, NB], BF16, tag=f"xb0_{t}")
        xb1 = xbig.tile([P, NB], BF16, tag=f"xb1_{t}")
        xres[0, t], xres[1, t] = xb0, xb1
        nc.sync.dma_start(xb0[:], x[0:P, t * NB : (t + 1) * NB])
        nc.sync.dma_start(xb1[:], x[P : 2 * P, t * NB : (t + 1) * NB])

        lt = lt_ps.tile([P, W_PER_T * K], F32, tag="lt")
        for g in range(G_PER_T):
            xt_p = xt_ps.tile([P, 4, C], BF16, tag="xtp")
            for wl in range(4):
                w = 4 * g + wl
                sl = slice(w * NW, (w + 1) * NW)
                # logitsT [128, 21]: x window chunk stationary, qk moving
                nc.tensor.matmul(
                    lt[:, w * K : (w + 1) * K], xb0[:, sl], qk0[:],
                    start=True, stop=False, skip_group_check=True,
                )
                nc.tensor.matmul(
                    lt[:, w * K : (w + 1) * K], xb1[:, sl], qk1[:],
                    start=False, stop=True, skip_group_check=True,
                )
                # xT window: [128c, 128n] -> [128n, 128c], bf16 PSUM
                nc.tensor.transpose(xt_p[:, wl, 0:P], xb0[:, sl], idb_sb[:])
                nc.tensor.transpose(xt_p[:, wl, P : 2 * P], xb1[:, sl], idb_sb[:])
            xt_s = xt_sb.tile([P, 4, C], F8, tag="xts")
            xts[t, g] = xt_s
            cp_eng[cp_i % 2](xt_s[:], xt_p[:])
            cp_i += 1

        pt = pt_sb.tile([P, W_PER_T, 32], F8, tag="pt")
        pts[t] = pt
        nc.scalar.activation(
            pt[:, :, 0:K], lt[:].rearrange("p (w k) -> p w k", k=K), AF.Exp
        )

        if t == N_TILES - 1:
            # phase-B consts: issue behind the whole x stream so they never
            # delay it (only needed once pass A finishes)
            nc.sync.dma_start(m2t0[:], m2[0:P, :])
            nc.sync.dma_start(m2t1[:], m2[P : 2 * P, :])
            nc.sync.dma_start(g0_sb[:], g0[:, :])
            nc.sync.dma_start(idf_sb[:], idf[:, :])

        if t > 0:
            emit_y(t - 1)
    emit_y(N_TILES - 1)

    # ---------------- Phase B: r -> xa -> gate (tiny) ------------------------
    y_sb = smalls.tile([K, C + 1], F32, tag="y_sb")
    nc.scalar.copy(y_sb[:], y[:])
    r_sb = smalls.tile([K, 1], F32, tag="r_sb")
    nc.vector.reciprocal(r_sb[:], y_sb[:, C : C + 1])

    # xa^T directly in column layout: xat[c, j] = sum_k y[k, c] * r[k]
    # (y_sb slice as stationary, r as 1-col moving) -- no row-xa, no transpose
    xat_ps = sm_ps.tile([P, 2], F32, tag="sm")
    for j in range(2):
        nc.tensor.matmul(
            xat_ps[:, j : j + 1], y_sb[:, j * P : (j + 1) * P], r_sb[:],
            start=True, stop=True, skip_group_check=True,
        )
    xat_sb = smalls.tile([P, 2], F32, tag="xat_sb")
    nc.vector.tensor_copy(xat_sb[:], xat_ps[:])

    gate_ps = sm_ps.tile([P, 2], F32, tag="sm")
    for cc in range(2):
        csl = slice(cc * P, (cc + 1) * P)
        nc.tensor.matmul(
            gate_ps[:, cc : cc + 1], m2t0[:, csl], xat_sb[:, 0:1],
            start=True, stop=False, skip_group_check=True,
        )
        nc.tensor.matmul(
            gate_ps[:, cc : cc + 1], m2t1[:, csl], xat_sb[:, 1:2],
            start=False, stop=True, skip_group_check=True,
        )
    gate_sb = smalls.tile([P, 2], F32, tag="gate_sb")
    nc.vector.tensor_add(gate_sb[:], gate_ps[:], g0_sb[:])

    # ---------------- Phase C: out = x * gate (in-place, stream out) ---------
    mul_rr = 0
    for t in range(N_TILES):
        for cc in range(2):
            xc = xres[cc, t]
            csl = slice(cc * P, (cc + 1) * P)
            nsl = slice(t * NB, (t + 1) * NB)
            if mul_rr % 3 == 2:
                nc.scalar.mul(xc[:], xc[:], gate_sb[:, cc : cc + 1])
            else:
                nc.vector.tensor_scalar_mul(xc[:], xc[:], gate_sb[:, cc : cc + 1])
            mul_rr += 1
            nc.sync.dma_start(out[csl, nsl], xc[:])


def build_nc(body=None):
    body = body or _body
    nc = bacc.Bacc(
        "TRN2",
        target_bir_lowering=False,
        debug=False,
        enable_asserts=False,
        num_devices=B,
    )
    x = nc.dram_tensor("x", [C, HW], BF16, kind="ExternalInput").ap()
    qk = nc.dram_tensor("qkT", [C, K], BF16, kind="ExternalInput").ap()
    m2 = nc.dram_tensor("m2t", [C, C], F32, kind="ExternalInput").ap()
    g0 = nc.dram_tensor("g0", [P, 2], F32, kind="ExternalInput").ap()
    idb = nc.dram_tensor("idb", [P, P], BF16, kind="ExternalInput").ap()
    idf = nc.dram_tensor("idf", [1, 1], F32, kind="ExternalInput").ap()
    out = nc.dram_tensor("out", [C, HW], BF16, kind="ExternalOutput").ap()

    with tile.TileContext(nc) as tc:
        with ExitStack() as ctx:
            body(ctx, tc, x, qk, m2, g0, idb, idf, out)
    nc.compile()
    return nc


_NC = None


def _get_nc():
    global _NC
    if _NC is None:
        _NC = build_nc()
    return _NC


def make_in_maps(x, query_embed, Wk, bk, Wv, bv, Wo, bo):
    x = np.asarray(x, dtype=np.float32)
    qe = np.asarray(query_embed, dtype=np.float64)
    Wk64 = np.asarray(Wk, dtype=np.float64)
    Wv64 = np.asarray(Wv, dtype=np.float64)
    Wo64 = np.asarray(Wo, dtype=np.float64)
    bv64 = np.asarray(bv, dtype=np.float64)
    bo64 = np.asarray(bo, dtype=np.float64)

    qkT = ((qe @ Wk64) / np.sqrt(float(D))).T.astype(BF).copy()
    m2t = (Wo64 @ Wv64).T.astype(np.float32).copy()
    g0 = (float(K) * (Wo64 @ bv64 + bo64)).astype(np.float32)
    g0c = np.ascontiguousarray(g0.reshape(2, P).T)
    idb = np.eye(P, dtype=np.float32).astype(BF)
    idf = np.ones((1, 1), dtype=np.float32)

    return [
        {
            "x": np.ascontiguousarray(x[b].reshape(C, HW)).astype(BF),
            "qkT": qkT,
            "m2t": m2t,
            "g0": g0c,
            "idb": idb,
            "idf": idf,
        }
        for b in range(B)
    ]


def kernel(x, query_embed, Wk, bk, Wv, bv, Wo, bo, _trace=False, **kw):
    in_maps = make_in_maps(x, query_embed, Wk, bk, Wv, bv, Wo, bo)
    nc = _get_nc()
    res = run_bass_kernel_spmd(nc, in_maps, core_ids=list(range(B)), trace=_trace, **kw)
    out = np.stack(
        [
            np.asarray(res.results[b]["out"], dtype=np.float32).reshape(C, 128, 128)
            for b in range(B)
        ]
    )
    if _trace:
        kernel.last_results = res
    return out



# revision 61
# speedup vs baseline: 1.0217x; 1.0217x over previous
"""Bass/Trainium2 kernel for nn_ClassQueryAttention.

Math (per batch b, x flattened to [C=256, N=16384]):
  logitsT[n,k] = x[:,n]^T qk[:,k]            (qk = (qe@Wk)^T/sqrt(D); per-k bias cancels in softmax)
  pT = exp(logitsT)                          (logits ~ N(0,1): no max-subtraction needed)
  y[k,c] = sum_n pT[n,k] x[c,n]  (flash-style PSUM accum; s[k] = sum_n pT[n,k] via ones column)
  xa[c]  = sum_k y[k,c]/s[k]
  gate   = (Wo@Wv) @ xa + K*(Wo@bv + bo)
  out[c,n] = x[c,n] * gate[c]

Single streamed pass: per 128-px window, logitsT comes from a matmul with the
x-chunk as the STATIONARY operand (21 moving cols instead of 256), so exp's
output is already in pT layout (no p transpose) at full 128-partition ACT
utilization. x is transposed per-window on the PE (bf16, 1 cyc/col); the
PSUM->SBUF copies convert xT to fp8e4 and alternate DVE/ACT. exp writes pT
as fp8e4 directly (padded to 32B/window for the DoubleRow weight-stride
rule). The y/s matmuls then run in fp8 DoubleRow mode: one matmul contracts
a PAIR of windows (lhsT [128,2,21], rhs [128,2,256]), halving PE time vs
bf16 (~10us saved/core; fp8 costs ~1e-2 rel err vs the 2e-2 gate, dominated
by xT quantization). y+s accumulate in one PSUM bank over all 128 windows,
so there is no softmax barrier; phase B is O(K*C) tiny, phase C multiplies
in-place and streams out.

Measured no-gos: DMA XBAR transpose runs ~100 GB/s on hw (5x slower than PE
transposes, and it head-of-line blocks its DMA queue); PE "pre-warm" dummy
matmuls and early/split tile-0 DMAs both LOSE time (the PE p-state ramp
rewards a late-but-dense schedule); compile-schedule variance between
near-identical programs is ~±4 us, so buffer counts (lt_ps=3, sm_ps=1) were
chosen by measurement.

HBM I/O is bf16 (host converts): 8 MiB in + 8 MiB out per core.
Sharding: data-parallel over batch B=8, one batch per NeuronCore, no collectives.
"""

import sys
from contextlib import ExitStack

import numpy as np
import ml_dtypes

sys.path.insert(0, "/opt/trn_rl_repo")

import concourse.bass as bass  # noqa: E402
import concourse.tile as tile  # noqa: E402
from concourse import bacc, mybir  # noqa: E402
from concourse.bass_utils import run_bass_kernel_spmd  # noqa: E402

B, C, HW = 8, 256, 128 * 128
K, D = 21, 256
P = 128          # partition count / channel chunk
NB = 2048        # big-tile pixels (per-tile DMA granularity)
NW = 128         # window pixels (transpose / pT granularity)
F32 = mybir.dt.float32
BF16 = mybir.dt.bfloat16
F8 = mybir.dt.float8e4
DR = mybir.MatmulPerfMode.DoubleRow
AF = mybir.ActivationFunctionType
BF = ml_dtypes.bfloat16

N_TILES = HW // NB           # 8
W_PER_T = NB // NW           # 16 windows per big tile
G_PER_T = W_PER_T // 4       # 4 transpose groups (4 windows each) per big tile
N_XBAR = 0                   # XBAR DMA transpose measured ~100GB/s on hw: off


def _body(ctx: ExitStack, tc: tile.TileContext, x, qk, m2, g0, idb, idf, out, sfx=""):
    nc = tc.nc

    def pool(name, **kw):
        return ctx.enter_context(tc.tile_pool(name=name + sfx, **kw))

    consts = pool("consts", bufs=1)
    qk0 = consts.tile([P, K], BF16, tag="qk0")
    qk1 = consts.tile([P, K], BF16, tag="qk1")
    m2t0 = consts.tile([P, C], F32, tag="m2t0")
    m2t1 = consts.tile([P, C], F32, tag="m2t1")
    g0_sb = consts.tile([P, 2], F32, tag="g0")
    idb_sb = consts.tile([P, P], BF16, tag="idb")
    idf_sb = consts.tile([1, 1], F32, tag="idf")


    # qk + idb are needed by the very first window's matmuls; m2t/g0/idf are
    # phase-B-only and issued after the x stream (end of pass A loop below).
    nc.sync.dma_start(qk0[:], qk[0:P, :])
    nc.sync.dma_start(qk1[:], qk[P : 2 * P, :])
    nc.sync.dma_start(idb_sb[:], idb[:, :])



    xbig = pool("xbig", bufs=1)
    lt_ps = pool("lt_ps", bufs=3, space="PSUM")     # logitsT [128, 16*21] f32
    xt_ps = pool("xt_ps", bufs=3, space="PSUM")     # xT group [128, 4, 256] bf16
    y_ps = pool("y_ps", bufs=1, space="PSUM")       # y+s accum [21, 257] f32
    pt_sb = pool("pt_sb", bufs=2)
    xt_sb = pool("xt_sb", bufs=8)
    smalls = pool("smalls", bufs=1)
    sm_ps = pool("sm_ps", bufs=1, space="PSUM")

    y = y_ps.tile([K, C + 1], F32, tag="y")
    nc.vector.memset(y[:], 0.0)

    # round-robin engine for the xT PSUM->SBUF copies (GPSIMD cannot read PSUM)
    cp_eng = [nc.vector.tensor_copy, nc.scalar.copy]
    cp_i = 0

    # xT tiles, manually rotated: each carries a fused ones column at
    # [.., .., C] so one DoubleRow matmul per window pair yields y AND the
    # softmax denominator s. Tile padded to C+8 for the DoubleRow pair-stride
    # rule. The ones memsets run on BOTH copy engines (DVE memset + ACT mul)
    # so each copy engine's program order places them before its first copy.
    xt_tiles = [
        xt_sb.tile([P, 4, C + 8], F8, tag=f"xts{i}", name=f"xts{i}")
        for i in range(8)
    ]
    for i, xt_i in enumerate(xt_tiles):
        nc.vector.memset(xt_i[:, :, C : C + 2], 1.0)
        nc.scalar.mul(xt_i[:, :, C : C + 1], xt_i[:, :, C : C + 1], 1.0)

    xres = {}
    pts = {}
    xts = {}

    def emit_y(t):
        """y/s matmuls for big tile t (pT + xT copies of tile t are complete).

        fp8 DoubleRow: each matmul contracts a PAIR of 128-px windows at
        0.5 cyc/output-row (lhsT [128, 2, K], rhs [128, 2, C])."""
        pt = pts[t]
        for w2 in range(W_PER_T // 2):
            gw2 = t * (W_PER_T // 2) + w2
            last = gw2 == HW // (2 * NW) - 1
            lhs = pt[:, 2 * w2 : 2 * w2 + 2, 0:K]
            rhs = xts[t, w2 // 2][:, 2 * (w2 % 2) : 2 * (w2 % 2) + 2, 0 : C + 1]
            # start=False always: the bank is pre-zeroed by the memset above
            # (separate start=True groups in one bank reset each other).
            nc.tensor.matmul(
                y[:], lhs, rhs,
                start=False, stop=last, perf_mode=DR, skip_group_check=True,
            )

    # ---------------- Pass A: stream tiles; accumulate y/s in PSUM ----------
    for t in range(N_TILES):
        xb0 = xbig.tile([P, NB], BF16, tag=f"xb0_{t}")
        xb1 = xbig.tile([P, NB], BF16, tag=f"xb1_{t}")
        xres[0, t], xres[1, t] = xb0, xb1
        nc.sync.dma_start(xb0[:], x[0:P, t * NB : (t + 1) * NB])
        nc.sync.dma_start(xb1[:], x[P : 2 * P, t * NB : (t + 1) * NB])

        lt = lt_ps.tile([P, W_PER_T * K], F32, tag="lt")
        for g in range(G_PER_T):
            xt_p = xt_ps.tile([P, 4, C], BF16, tag="xtp")
            for wl in range(4):
                w = 4 * g + wl
                sl = slice(w * NW, (w + 1) * NW)
                # logitsT [128, 21]: x window chunk stationary, qk moving
                nc.tensor.matmul(
                    lt[:, w * K : (w + 1) * K], xb0[:, sl], qk0[:],
                    start=True, stop=False, skip_group_check=True,
                )
                nc.tensor.matmul(
                    lt[:, w * K : (w + 1) * K], xb1[:, sl], qk1[:],
                    start=False, stop=True, skip_group_check=True,
                )
                # xT window: [128c, 128n] -> [128n, 128c], bf16 PSUM
                nc.tensor.transpose(xt_p[:, wl, 0:P], xb0[:, sl], idb_sb[:])
                nc.tensor.transpose(xt_p[:, wl, P : 2 * P], xb1[:, sl], idb_sb[:])
            xt_s = xt_tiles[(t * G_PER_T + g) % 8]
            xts[t, g] = xt_s
            cp_eng[cp_i % 2](xt_s[:, :, 0:C], xt_p[:])
            cp_i += 1

        pt = pt_sb.tile([P, W_PER_T, 32], F8, tag="pt")
        pts[t] = pt
        nc.scalar.activation(
            pt[:, :, 0:K], lt[:].rearrange("p (w k) -> p w k", k=K), AF.Exp
        )

        if t == N_TILES - 1:
            # phase-B consts: issue behind the whole x stream so they never
            # delay it (only needed once pass A finishes)
            nc.sync.dma_start(m2t0[:], m2[0:P, :])
            nc.sync.dma_start(m2t1[:], m2[P : 2 * P, :])
            nc.sync.dma_start(g0_sb[:], g0[:, :])
            nc.sync.dma_start(idf_sb[:], idf[:, :])

        if t > 0:
            emit_y(t - 1)
    emit_y(N_TILES - 1)

    # ---------------- Phase B: r -> xa -> gate (tiny) ------------------------
    y_sb = smalls.tile([K, C + 1], F32, tag="y_sb")
    nc.scalar.copy(y_sb[:], y[:])
    r_sb = smalls.tile([K, 1], F32, tag="r_sb")
    nc.vector.reciprocal(r_sb[:], y_sb[:, C : C + 1])

    # xa^T directly in column layout: xat[c, j] = sum_k y[k, c] * r[k]
    # (y_sb slice as stationary, r as 1-col moving) -- no row-xa, no transpose
    xat_ps = sm_ps.tile([P, 2], F32, tag="sm")
    for j in range(2):
        nc.tensor.matmul(
            xat_ps[:, j : j + 1], y_sb[:, j * P : (j + 1) * P], r_sb[:],
            start=True, stop=True, skip_group_check=True,
        )
    xat_sb = smalls.tile([P, 2], F32, tag="xat_sb")
    nc.vector.tensor_copy(xat_sb[:], xat_ps[:])

    gate_ps = sm_ps.tile([P, 2], F32, tag="sm")
    for cc in range(2):
        csl = slice(cc * P, (cc + 1) * P)
        nc.tensor.matmul(
            gate_ps[:, cc : cc + 1], m2t0[:, csl], xat_sb[:, 0:1],
            start=True, stop=False, skip_group_check=True,
        )
        nc.tensor.matmul(
            gate_ps[:, cc : cc + 1], m2t1[:, csl], xat_sb[:, 1:2],
            start=False, stop=True, skip_group_check=True,
        )
    gate_sb = smalls.tile([P, 2], F32, tag="gate_sb")
    nc.vector.tensor_add(gate_sb[:], gate_ps[:], g0_sb[:])

    # ---------------- Phase C: out = x * gate (in-place, stream out) ---------
    mul_rr = 0
    for t in range(N_TILES):
        for cc in range(2):
            xc = xres[cc, t]
            csl = slice(cc * P, (cc + 1) * P)
            nsl = slice(t * NB, (t + 1) * NB)
            if mul_rr % 3 == 2:
                nc.scalar.mul(xc[:], xc[:], gate_sb[:, cc : cc + 1])
            else:
                nc.vector.tensor_scalar_mul(xc[:], xc[:], gate_sb[:, cc : cc + 1])
            mul_rr += 1
            nc.sync.dma_start(out[csl, nsl], xc[:])


def build_nc(body=None):
    body = body or _body
    nc = bacc.Bacc(
        "TRN2",
        target_bir_lowering=False,
        debug=False,
        enable_asserts=False,
        num_devices=B,
    )
    x = nc.dram_tensor("x", [C, HW], BF16, kind="ExternalInput").ap()
    qk = nc.dram_tensor("qkT", [C, K], BF16, kind="ExternalInput").ap()
    m2 = nc.dram_tensor("m2t", [C, C], F32, kind="ExternalInput").ap()
    g0 = nc.dram_tensor("g0", [P, 2], F32, kind="ExternalInput").ap()
    idb = nc.dram_tensor("idb", [P, P], BF16, kind="ExternalInput").ap()
    idf = nc.dram_tensor("idf", [1, 1], F32, kind="ExternalInput").ap()
    out = nc.dram_tensor("out", [C, HW], BF16, kind="ExternalOutput").ap()

    with tile.TileContext(nc) as tc:
        with ExitStack() as ctx:
            body(ctx, tc, x, qk, m2, g0, idb, idf, out)
    nc.compile()
    return nc


_NC = None


def _get_nc():
    global _NC
    if _NC is None:
        _NC = build_nc()
    return _NC


def make_in_maps(x, query_embed, Wk, bk, Wv, bv, Wo, bo):
    x = np.asarray(x, dtype=np.float32)
    qe = np.asarray(query_embed, dtype=np.float64)
    Wk64 = np.asarray(Wk, dtype=np.float64)
    Wv64 = np.asarray(Wv, dtype=np.float64)
    Wo64 = np.asarray(Wo, dtype=np.float64)
    bv64 = np.asarray(bv, dtype=np.float64)
    bo64 = np.asarray(bo, dtype=np.float64)

    qkT = ((qe @ Wk64) / np.sqrt(float(D))).T.astype(BF).copy()
    m2t = (Wo64 @ Wv64).T.astype(np.float32).copy()
    g0 = (float(K) * (Wo64 @ bv64 + bo64)).astype(np.float32)
    g0c = np.ascontiguousarray(g0.reshape(2, P).T)
    idb = np.eye(P, dtype=np.float32).astype(BF)
    idf = np.ones((1, 1), dtype=np.float32)

    return [
        {
            "x": np.ascontiguousarray(x[b].reshape(C, HW)).astype(BF),
            "qkT": qkT,
            "m2t": m2t,
            "g0": g0c,
            "idb": idb,
            "idf": idf,
        }
        for b in range(B)
    ]


def kernel(x, query_embed, Wk, bk, Wv, bv, Wo, bo, _trace=False, **kw):
    in_maps = make_in_maps(x, query_embed, Wk, bk, Wv, bv, Wo, bo)
    nc = _get_nc()
    res = run_bass_kernel_spmd(nc, in_maps, core_ids=list(range(B)), trace=_trace, **kw)
    out = np.stack(
        [
            np.asarray(res.results[b]["out"], dtype=np.float32).reshape(C, 128, 128)
            for b in range(B)
        ]
    )
    if _trace:
        kernel.last_results = res
    return out



# revision 62
# speedup vs baseline: 1.0771x; 1.0542x over previous
"""Bass/Trainium2 kernel for nn_ClassQueryAttention.

Math (per batch b, x flattened to [C=256, N=16384]):
  logitsT[n,k] = x[:,n]^T qk[:,k]            (qk = (qe@Wk)^T/sqrt(D); per-k bias cancels in softmax)
  pT = exp(logitsT)                          (logits ~ N(0,1): no max-subtraction needed)
  y[k,c] = sum_n pT[n,k] x[c,n]  (flash-style PSUM accum; s[k] = sum_n pT[n,k] via ones column)
  xa[c]  = sum_k y[k,c]/s[k]
  gate   = (Wo@Wv) @ xa + K*(Wo@bv + bo)
  out[c,n] = x[c,n] * gate[c]

Single streamed pass: per 128-px window, logitsT comes from a matmul with the
x-chunk as the STATIONARY operand (21 moving cols instead of 256), so exp's
output is already in pT layout (no p transpose) at full 128-partition ACT
utilization. x is transposed per-window on the PE (bf16, 1 cyc/col); the
PSUM->SBUF copies convert xT to fp8e4 and alternate DVE/ACT. exp writes pT
as fp8e4 directly (padded to 32B/window for the DoubleRow weight-stride
rule). The y/s matmuls then run in fp8 DoubleRow mode: one matmul contracts
a PAIR of windows (lhsT [128,2,21], rhs [128,2,257]), halving PE time vs
bf16 (~10us saved/core; fp8 costs ~1e-2 rel err vs the 2e-2 gate, dominated
by xT quantization). The xT tiles carry a fused ones column at [..,C] so the
same matmul also accumulates the softmax denominator s; the ones column is
written via BOTH copy engines' queues (DVE memset + ACT 1.0-mul) because a
memset alone has no tracker ordering vs the disjoint [0:C] copies and raced
on hw (NaN). y+s accumulate in one PSUM bank over all 128 windows, so there
is no softmax barrier; phase B computes xa^T directly in column layout
(y_sb chunks stationary x r moving), phase C multiplies in-place and
streams out.

Measured no-gos: DMA XBAR transpose runs ~100 GB/s on hw (5x slower than PE
transposes, and it head-of-line blocks its DMA queue); PE "pre-warm" dummy
matmuls and early/split tile-0 DMAs both LOSE time (the PE p-state ramp
rewards a late-but-dense schedule); compile-schedule variance between
near-identical programs is ~±4 us, so buffer counts (lt_ps=3, sm_ps=1) were
chosen by measurement.

HBM I/O is bf16 (host converts): 8 MiB in + 8 MiB out per core.
Sharding: data-parallel over batch B=8, one batch per NeuronCore, no collectives.
"""

import sys
from contextlib import ExitStack

import numpy as np
import ml_dtypes

sys.path.insert(0, "/opt/trn_rl_repo")

import concourse.bass as bass  # noqa: E402
import concourse.tile as tile  # noqa: E402
from concourse import bacc, mybir  # noqa: E402
from concourse.bass_utils import run_bass_kernel_spmd  # noqa: E402

B, C, HW = 8, 256, 128 * 128
K, D = 21, 256
P = 128          # partition count / channel chunk
NB = 2048        # big-tile pixels (per-tile DMA granularity)
NW = 128         # window pixels (transpose / pT granularity)
F32 = mybir.dt.float32
BF16 = mybir.dt.bfloat16
F8 = mybir.dt.float8e4
DR = mybir.MatmulPerfMode.DoubleRow
AF = mybir.ActivationFunctionType
BF = ml_dtypes.bfloat16

N_TILES = HW // NB           # 8
W_PER_T = NB // NW           # 16 windows per big tile
G_PER_T = W_PER_T // 4       # 4 transpose groups (4 windows each) per big tile
N_XBAR = 0                   # XBAR DMA transpose measured ~100GB/s on hw: off


def _body(ctx: ExitStack, tc: tile.TileContext, x, qk, m2, g0, idb, idf, out, sfx=""):
    nc = tc.nc

    def pool(name, **kw):
        return ctx.enter_context(tc.tile_pool(name=name + sfx, **kw))

    consts = pool("consts", bufs=1)
    qk0 = consts.tile([P, K], BF16, tag="qk0")
    qk1 = consts.tile([P, K], BF16, tag="qk1")
    m2t0 = consts.tile([P, C], F32, tag="m2t0")
    m2t1 = consts.tile([P, C], F32, tag="m2t1")
    g0_sb = consts.tile([P, 2], F32, tag="g0")
    idb_sb = consts.tile([P, P], BF16, tag="idb")
    idf_sb = consts.tile([1, 1], F32, tag="idf")


    # qk + idb are needed by the very first window's matmuls; m2t/g0/idf are
    # phase-B-only and issued after the x stream (end of pass A loop below).
    nc.sync.dma_start(qk0[:], qk[0:P, :])
    nc.sync.dma_start(qk1[:], qk[P : 2 * P, :])
    nc.sync.dma_start(idb_sb[:], idb[:, :])



    xbig = pool("xbig", bufs=1)
    lt_ps = pool("lt_ps", bufs=3, space="PSUM")     # logitsT [128, 16*21] f32
    xt_ps = pool("xt_ps", bufs=3, space="PSUM")     # xT group [128, 4, 256] bf16
    y_ps = pool("y_ps", bufs=1, space="PSUM")       # y+s accum [21, 257] f32
    pt_sb = pool("pt_sb", bufs=2)
    xt_sb = pool("xt_sb", bufs=8)
    smalls = pool("smalls", bufs=1)
    sm_ps = pool("sm_ps", bufs=1, space="PSUM")

    y = y_ps.tile([K, C + 1], F32, tag="y")
    nc.vector.memset(y[:], 0.0)

    # round-robin engine for the xT PSUM->SBUF copies (GPSIMD cannot read PSUM)
    cp_eng = [nc.vector.tensor_copy, nc.scalar.copy]
    cp_i = 0

    # xT tiles, manually rotated: each carries a fused ones column at
    # [.., .., C] so one DoubleRow matmul per window pair yields y AND the
    # softmax denominator s. Tile padded to C+8 for the DoubleRow pair-stride
    # rule. The ones memsets run on BOTH copy engines (DVE memset + ACT mul)
    # so each copy engine's program order places them before its first copy.
    xt_tiles = [
        xt_sb.tile([P, 4, C + 8], F8, tag=f"xts{i}", name=f"xts{i}")
        for i in range(8)
    ]
    for i, xt_i in enumerate(xt_tiles):
        nc.vector.memset(xt_i[:, :, C : C + 2], 1.0)
        nc.scalar.mul(xt_i[:, :, C : C + 1], xt_i[:, :, C : C + 1], 1.0)

    xres = {}
    pts = {}
    xts = {}

    def emit_y(t):
        """y/s matmuls for big tile t (pT + xT copies of tile t are complete).

        fp8 DoubleRow: each matmul contracts a PAIR of 128-px windows at
        0.5 cyc/output-row (lhsT [128, 2, K], rhs [128, 2, C])."""
        pt = pts[t]
        for w2 in range(W_PER_T // 2):
            gw2 = t * (W_PER_T // 2) + w2
            last = gw2 == HW // (2 * NW) - 1
            lhs = pt[:, 2 * w2 : 2 * w2 + 2, 0:K]
            rhs = xts[t, w2 // 2][:, 2 * (w2 % 2) : 2 * (w2 % 2) + 2, 0 : C + 1]
            # start=False always: the bank is pre-zeroed by the memset above
            # (separate start=True groups in one bank reset each other).
            nc.tensor.matmul(
                y[:], lhs, rhs,
                start=False, stop=last, perf_mode=DR, skip_group_check=True,
            )

    # ---------------- Pass A: stream tiles; accumulate y/s in PSUM ----------
    for t in range(N_TILES):
        xb0 = xbig.tile([P, NB], BF16, tag=f"xb0_{t}")
        xb1 = xbig.tile([P, NB], BF16, tag=f"xb1_{t}")
        xres[0, t], xres[1, t] = xb0, xb1
        nc.sync.dma_start(xb0[:], x[0:P, t * NB : (t + 1) * NB])
        nc.sync.dma_start(xb1[:], x[P : 2 * P, t * NB : (t + 1) * NB])

        lt = lt_ps.tile([P, W_PER_T * K], F32, tag="lt")
        for g in range(G_PER_T):
            xt_p = xt_ps.tile([P, 4, C], BF16, tag="xtp")
            for wl in range(4):
                w = 4 * g + wl
                sl = slice(w * NW, (w + 1) * NW)
                # logitsT [128, 21]: x window chunk stationary, qk moving
                nc.tensor.matmul(
                    lt[:, w * K : (w + 1) * K], xb0[:, sl], qk0[:],
                    start=True, stop=False, skip_group_check=True,
                )
                nc.tensor.matmul(
                    lt[:, w * K : (w + 1) * K], xb1[:, sl], qk1[:],
                    start=False, stop=True, skip_group_check=True,
                )
                # xT window: [128c, 128n] -> [128n, 128c], bf16 PSUM
                nc.tensor.transpose(xt_p[:, wl, 0:P], xb0[:, sl], idb_sb[:])
                nc.tensor.transpose(xt_p[:, wl, P : 2 * P], xb1[:, sl], idb_sb[:])
            xt_s = xt_tiles[(t * G_PER_T + g) % 8]
            xts[t, g] = xt_s
            cp_eng[cp_i % 2](xt_s[:, :, 0:C], xt_p[:])
            cp_i += 1

        pt = pt_sb.tile([P, W_PER_T, 32], F8, tag="pt")
        pts[t] = pt
        nc.scalar.activation(
            pt[:, :, 0:K], lt[:].rearrange("p (w k) -> p w k", k=K), AF.Exp
        )

        if t == N_TILES - 1:
            # phase-B consts: issue behind the whole x stream so they never
            # delay it (only needed once pass A finishes)
            nc.sync.dma_start(m2t0[:], m2[0:P, :])
            nc.sync.dma_start(m2t1[:], m2[P : 2 * P, :])
            nc.sync.dma_start(g0_sb[:], g0[:, :])
            nc.sync.dma_start(idf_sb[:], idf[:, :])

        if t > 0:
            emit_y(t - 1)
    emit_y(N_TILES - 1)

    # ---------------- Phase B: r -> xa -> gate (tiny) ------------------------
    y_sb = smalls.tile([K, C + 1], F32, tag="y_sb")
    nc.scalar.copy(y_sb[:], y[:])
    r_sb = smalls.tile([K, 1], F32, tag="r_sb")
    nc.vector.reciprocal(r_sb[:], y_sb[:, C : C + 1])

    # xa^T directly in column layout: xat[c, j] = sum_k y[k, c] * r[k]
    # (y_sb slice as stationary, r as 1-col moving) -- no row-xa, no transpose
    xat_ps = sm_ps.tile([P, 2], F32, tag="sm")
    for j in range(2):
        nc.tensor.matmul(
            xat_ps[:, j : j + 1], y_sb[:, j * P : (j + 1) * P], r_sb[:],
            start=True, stop=True, skip_group_check=True,
        )
    xat_sb = smalls.tile([P, 2], F32, tag="xat_sb")
    nc.vector.tensor_copy(xat_sb[:], xat_ps[:])

    gate_ps = sm_ps.tile([P, 2], F32, tag="sm")
    for cc in range(2):
        csl = slice(cc * P, (cc + 1) * P)
        nc.tensor.matmul(
            gate_ps[:, cc : cc + 1], m2t0[:, csl], xat_sb[:, 0:1],
            start=True, stop=False, skip_group_check=True,
        )
        nc.tensor.matmul(
            gate_ps[:, cc : cc + 1], m2t1[:, csl], xat_sb[:, 1:2],
            start=False, stop=True, skip_group_check=True,
        )
    gate_sb = smalls.tile([P, 2], F32, tag="gate_sb")
    nc.vector.tensor_add(gate_sb[:], gate_ps[:], g0_sb[:])

    # ---------------- Phase C: out = x * gate (in-place, stream out) ---------
    mul_rr = 0
    for t in range(N_TILES):
        for cc in range(2):
            xc = xres[cc, t]
            csl = slice(cc * P, (cc + 1) * P)
            nsl = slice(t * NB, (t + 1) * NB)
            if mul_rr % 3 == 2:
                nc.scalar.mul(xc[:], xc[:], gate_sb[:, cc : cc + 1])
            else:
                nc.vector.tensor_scalar_mul(xc[:], xc[:], gate_sb[:, cc : cc + 1])
            mul_rr += 1
            nc.sync.dma_start(out[csl, nsl], xc[:])


def build_nc(body=None):
    body = body or _body
    nc = bacc.Bacc(
        "TRN2",
        target_bir_lowering=False,
        debug=False,
        enable_asserts=False,
        num_devices=B,
    )
    x = nc.dram_tensor("x", [C, HW], BF16, kind="ExternalInput").ap()
    qk = nc.dram_tensor("qkT", [C, K], BF16, kind="ExternalInput").ap()
    m2 = nc.dram_tensor("m2t", [C, C], F32, kind="ExternalInput").ap()
    g0 = nc.dram_tensor("g0", [P, 2], F32, kind="ExternalInput").ap()
    idb = nc.dram_tensor("idb", [P, P], BF16, kind="ExternalInput").ap()
    idf = nc.dram_tensor("idf", [1, 1], F32, kind="ExternalInput").ap()
    out = nc.dram_tensor("out", [C, HW], BF16, kind="ExternalOutput").ap()

    with tile.TileContext(nc) as tc:
        with ExitStack() as ctx:
            body(ctx, tc, x, qk, m2, g0, idb, idf, out)
    nc.compile()
    return nc


_NC = None


def _get_nc():
    global _NC
    if _NC is None:
        _NC = build_nc()
    return _NC


def make_in_maps(x, query_embed, Wk, bk, Wv, bv, Wo, bo):
    x = np.asarray(x, dtype=np.float32)
    qe = np.asarray(query_embed, dtype=np.float64)
    Wk64 = np.asarray(Wk, dtype=np.float64)
    Wv64 = np.asarray(Wv, dtype=np.float64)
    Wo64 = np.asarray(Wo, dtype=np.float64)
    bv64 = np.asarray(bv, dtype=np.float64)
    bo64 = np.asarray(bo, dtype=np.float64)

    qkT = ((qe @ Wk64) / np.sqrt(float(D))).T.astype(BF).copy()
    m2t = (Wo64 @ Wv64).T.astype(np.float32).copy()
    g0 = (float(K) * (Wo64 @ bv64 + bo64)).astype(np.float32)
    g0c = np.ascontiguousarray(g0.reshape(2, P).T)
    idb = np.eye(P, dtype=np.float32).astype(BF)
    idf = np.ones((1, 1), dtype=np.float32)

    return [
        {
            "x": np.ascontiguousarray(x[b].reshape(C, HW)).astype(BF),
            "qkT": qkT,
            "m2t": m2t,
            "g0": g0c,
            "idb": idb,
            "idf": idf,
        }
        for b in range(B)
    ]


def kernel(x, query_embed, Wk, bk, Wv, bv, Wo, bo, _trace=False, **kw):
    in_maps = make_in_maps(x, query_embed, Wk, bk, Wv, bv, Wo, bo)
    nc = _get_nc()
    res = run_bass_kernel_spmd(nc, in_maps, core_ids=list(range(B)), trace=_trace, **kw)
    out = np.stack(
        [
            np.asarray(res.results[b]["out"], dtype=np.float32).reshape(C, 128, 128)
            for b in range(B)
        ]
    )
    if _trace:
        kernel.last_results = res
    return out

